# revision 6
# baseline (speedup 1.0000x reference)
"""Sparse-attention kernel for 8 trn2 NeuronCores (Bass/Tile).

Math (reference):
    Q = x1 @ Wq.T + bq                       [N1, DIM]
    K = x2 @ Wk.T + bk                       [N2, DIM]
    scores = (Q @ K.T) / sqrt(ITEM)          [N1, N2]
    e = exp(scores) * label_map
    att = e / (sum_j e + 1e-8) * (sum_j label_map / topk + 1e-8)
    out = att @ x2                           [N1, ITEM]

Key transformations used here:
  * Rows of x1/label_map are sharded across 8 cores (512 rows each).
  * scores = Q @ K.T is reassociated as (Q @ Wk) @ x2.T + (Q @ bk) 1^T.
    The (Q @ bk) term is constant per output row i; it scales both the
    numerator e and the denominator sum(e) by exp(c_i), which cancels in
    the normalization (the +1e-8 epsilon makes this inexact only at the
    ~1e-11 relative level since sum(e) is O(1e3)).  So bk drops out and no
    core ever computes the K projection (saving 33.5 GFLOP/core).
  * The 1/sqrt(ITEM) scale and bq bias are folded into the Q epilogue.
  * The per-row normalization a_i is applied to the final out rows, so the
    unnormalized e.T tiles (built via PE transposes) feed the spmm directly.
  * Matmul operands are bf16 (fp32 PSUM accumulation).  bf16 weights get
    the fast-weight-load path; 4-byte weights serialize LDWEIGHTS with the
    matmul (~+190 ns each).
  * Every DRAM stream is host-rearranged partition-major so each SBUF slab
    loads with ONE fully contiguous DMA (multi-KB lines on both sides), and
    matmul operand tiles are free-dim slices of resident slabs.
"""

import math
import os

import numpy as np

# Walrus pages DRAM tensors at --dram-page-size bytes (fed by this env var);
# the default of 256 fragments every DMA into 256-byte packets.  The same
# variable is read as *megabytes* for the NRT scratchpad page (cap 4096).
os.environ.setdefault("NEURON_SCRATCHPAD_PAGE_SIZE", "4096")

try:
    import concourse.bass as bass
except ImportError:  # fresh interpreter without the boot path
    import sys

    sys.path.insert(0, "/opt/trn_rl_repo")
    import concourse.bass as bass

import ml_dtypes
import concourse.mybir as mybir
import concourse.tile as tile
from concourse import bacc
from concourse.bass_utils import run_bass_kernel_spmd
from concourse.masks import make_identity

NCORES = 8
F32 = mybir.dt.float32
BF16 = mybir.dt.bfloat16
NPBF16 = ml_dtypes.bfloat16


def _build(S, N2, ITEM, DIMP, denom, topk_f):
    """Build the per-core Bass program.

    S     - x1 rows per core (multiple of 128)
    N2    - x2 rows (multiple of 512)
    ITEM  - feature dim (multiple of 512)
    DIMP  - projection dim padded to a multiple of 128
    denom - sqrt(original ITEM)
    """
    IC = S // 128  # output-row chunks
    JC = N2 // 128  # x2-row chunks (spmm contraction)
    JN = N2 // 512  # 512-wide tiles of the scores free dim
    TC = ITEM // 128  # feature chunks (scores contraction)
    TN = ITEM // 512  # 512-wide tiles of the output free dim
    DC = DIMP // 128  # projection-dim chunks
    Exp = mybir.ActivationFunctionType.Exp
    Mult = mybir.AluOpType.mult
    Add = mybir.AluOpType.add
    X = mybir.AxisListType.X

    nc = bacc.Bacc("TRN2", target_bir_lowering=False, debug=False, num_devices=NCORES)
    # All streams are partition-major slabs: [slab_idx, 128, inner...] where
    # the per-partition inner block is contiguous in DRAM.
    x1t = nc.dram_tensor("x1t", [128, TC, S], BF16, kind="ExternalInput")
    wqt = nc.dram_tensor("wqt", [DC, 128, TC, 128], BF16, kind="ExternalInput")
    wk = nc.dram_tensor("wk", [TC, 128, DC, 128], BF16, kind="ExternalInput")
    x2t = nc.dram_tensor("x2t", [JN, 128, TC, 512], BF16, kind="ExternalInput")
    x2n = nc.dram_tensor("x2n", [TN, 128, JC, 512], BF16, kind="ExternalInput")
    lm = nc.dram_tensor("lm", [JN, 128, IC, 512], BF16, kind="ExternalInput")
    bq2 = nc.dram_tensor("bq2", [128, DC], F32, kind="ExternalInput")
    y = nc.dram_tensor("y", [S, ITEM], F32, kind="ExternalOutput")

    with tile.TileContext(nc) as tc:
        with (
            tc.tile_pool(name="big", bufs=1) as big,
            tc.tile_pool(name="persist", bufs=1) as persist,
            tc.tile_pool(name="stream", bufs=4) as stream,
            tc.tile_pool(name="slab", bufs=2) as slabpool,
            tc.tile_pool(name="wq", bufs=2) as wqpool,
            tc.tile_pool(name="wkp", bufs=2) as wkpool,
            tc.tile_pool(name="lmp", bufs=2) as lmpool,
            tc.tile_pool(name="acc", bufs=4, space="PSUM") as accp,
            tc.tile_pool(name="trp", bufs=4, space="PSUM") as trp,
        ):
            ident = persist.tile([128, 128], BF16, tag="ident")
            make_identity(nc, ident[:])
            zbias = persist.tile([128, 1], F32, tag="zbias")
            nc.gpsimd.memset(zbias[:], 0.0)
            bqt = persist.tile([128, DC], F32, tag="bqt")
            nc.sync.dma_start(bqt[:], bq2[:])
            bqs = persist.tile([128, DC], F32, tag="bqs")
            nc.vector.tensor_scalar_mul(bqs[:], bqt[:], 1.0 / denom)

            # phase 1: QT[d, i] = (x1 @ Wq.T + bq) / denom, DIM-major
            x1t_s = big.tile([128, TC, S], BF16, tag="bigA")
            nc.sync.dma_start(x1t_s[:], x1t[:])
            qt_s = persist.tile([128, DC, S], BF16, tag="qt")
            for d in range(DC):
                wsl = wqpool.tile([128, TC, 128], BF16, tag="wq")
                nc.sync.dma_start(wsl[:], wqt[d])
                ps = accp.tile([128, 512], F32, tag="acc")
                for t in range(TC):
                    nc.tensor.matmul(
                        ps[:, :S],
                        wsl[:, t, :],
                        x1t_s[:, t, :],
                        start=(t == 0),
                        stop=(t == TC - 1),
                    )
                nc.vector.tensor_scalar(
                    qt_s[:, d, :], ps[:, :S], 1.0 / denom, bqs[:, d : d + 1],
                    op0=Mult, op1=Add,
                )

            # phase 2: AT[t, i] = sum_d Wk[d, t] * QT[d, i]   (= (Q @ Wk).T)
            at_s = big.tile([128, TC, S], BF16, tag="bigB")
            for t in range(TC):
                wsl = wkpool.tile([128, DC, 128], BF16, tag="wk")
                nc.sync.dma_start(wsl[:], wk[t])
                ps = accp.tile([128, 512], F32, tag="acc")
                for d in range(DC):
                    nc.tensor.matmul(
                        ps[:, :S],
                        wsl[:, d, :],
                        qt_s[:, d, :],
                        start=(d == 0),
                        stop=(d == DC - 1),
                    )
                nc.scalar.copy(at_s[:, t, :], ps[:, :S])

            # phase 3: scores -> exp -> *label -> row-sums -> transpose to eT
            et_s = big.tile([128, JC, IC * 128], BF16, tag="bigA")
            s_parts = persist.tile([128, IC, JN], F32, tag="sparts")
            i_parts = persist.tile([128, IC, JN], F32, tag="iparts")
            for jn in range(JN):
                xsl = slabpool.tile([128, TC, 512], BF16, tag="slab")
                nc.sync.dma_start(xsl[:], x2t[jn])
                lsl = lmpool.tile([128, IC, 512], BF16, tag="lmt")
                nc.sync.dma_start(lsl[:], lm[jn])
                for i in range(IC):
                    ps = accp.tile([128, 512], F32, tag="acc", name=f"ps3_{jn}_{i}")
                    for t in range(TC):
                        nc.tensor.matmul(
                            ps[:],
                            at_s[:, t, i * 128 : (i + 1) * 128],
                            xsl[:, t, :],
                            start=(t == 0),
                            stop=(t == TC - 1),
                        )
                    e = stream.tile([128, 512], BF16, tag="e")
                    nc.scalar.activation(e[:], ps[:], Exp, bias=zbias[:])
                    nc.vector.reduce_sum(
                        i_parts[:, i, jn : jn + 1], lsl[:, i, :], axis=X
                    )
                    nc.vector.tensor_mul(e[:], e[:], lsl[:, i, :])
                    nc.vector.reduce_sum(s_parts[:, i, jn : jn + 1], e[:], axis=X)
                    for jj in range(4):
                        pt = trp.tile([128, 128], BF16, tag="tr")
                        nc.tensor.transpose(
                            pt[:], e[:, jj * 128 : (jj + 1) * 128], ident[:]
                        )
                        nc.scalar.copy(
                            et_s[:, jn * 4 + jj, i * 128 : (i + 1) * 128], pt[:]
                        )

            # a_i = (interactions/topk + 1e-8) / (sum_e + 1e-8)
            s_all = persist.tile([128, IC, 1], F32, tag="sall")
            nc.vector.reduce_sum(s_all[:], s_parts[:], axis=X)
            nc.vector.tensor_scalar_add(s_all[:], s_all[:], 1e-8)
            rec = persist.tile([128, IC, 1], F32, tag="rec")
            nc.vector.reciprocal(rec[:], s_all[:])
            i_all = persist.tile([128, IC, 1], F32, tag="iall")
            nc.vector.reduce_sum(i_all[:], i_parts[:], axis=X)
            nc.vector.tensor_scalar(
                i_all[:], i_all[:], 1.0 / topk_f, 1e-8, op0=Mult, op1=Add
            )
            a_all = persist.tile([128, IC, 1], F32, tag="aall")
            nc.vector.tensor_mul(a_all[:], i_all[:], rec[:])

            # phase 4: out[i, :] = a_i * sum_j eT[j, i] * x2[j, :]
            for n in range(TN):
                xsl = slabpool.tile([128, JC, 512], BF16, tag="slab")
                nc.sync.dma_start(xsl[:], x2n[n])
                for i in range(IC):
                    ps = accp.tile([128, 512], F32, tag="acc", name=f"ps4_{n}_{i}")
                    for j in range(JC):
                        nc.tensor.matmul(
                            ps[:],
                            et_s[:, j, i * 128 : (i + 1) * 128],
                            xsl[:, j, :],
                            start=(j == 0),
                            stop=(j == JC - 1),
                        )
                    o = stream.tile([128, 512], F32, tag="osb")
                    nc.vector.tensor_scalar_mul(o[:], ps[:], a_all[:, i, :])
                    nc.sync.dma_start(
                        y[i * 128 : (i + 1) * 128, n * 512 : (n + 1) * 512], o[:]
                    )

    nc.compile()
    return nc


def _pmajor(a, p, inner):
    """[R, C] with R = nblk*p -> [p, nblk, inner...] partition-major, where
    each partition's inner block is contiguous."""
    R, C = a.shape
    nblk = R // p
    return np.ascontiguousarray(a.reshape(nblk, p, C).transpose(1, 0, 2))


def _in_maps(x1, x2, label_map, Wq, bq, Wk, DIMP, S):
    ITEM = x1.shape[1]
    N2 = x2.shape[0]
    DIM = Wq.shape[0]
    DC = DIMP // 128
    TC = ITEM // 128
    JN = N2 // 512
    TN = ITEM // 512
    JC = N2 // 128
    IC = S // 128

    wqp = np.zeros((DIMP, ITEM), NPBF16)
    wqp[:DIM] = Wq.astype(NPBF16)
    wkp = np.zeros((DIMP, ITEM), NPBF16)
    wkp[:DIM] = Wk.astype(NPBF16)
    bqp = np.zeros((DIMP,), np.float32)
    bqp[:DIM] = bq
    bq2 = np.ascontiguousarray(bqp.reshape(DC, 128).T)

    x1b = x1.astype(NPBF16)
    x2b = x2.astype(NPBF16)
    wqT = np.ascontiguousarray(wqp.T)  # [ITEM, DIMP]
    x2T = np.ascontiguousarray(x2b.T)  # [ITEM, N2]

    # wqt[d] = WqT[:, d-chunk] as [128, TC, 128] partition-major
    wqt = np.stack(
        [_pmajor(wqT[:, d * 128 : (d + 1) * 128], 128, None) for d in range(DC)]
    )
    # wk[t] = Wk_pad[:, t-chunk] as [128, DC, 128]
    wkt = np.stack(
        [_pmajor(wkp[:, t * 128 : (t + 1) * 128], 128, None) for t in range(TC)]
    )
    # x2t[jn] = x2T[:, jn-chunk] as [128, TC, 512]
    x2tb = np.stack(
        [_pmajor(x2T[:, j * 512 : (j + 1) * 512], 128, None) for j in range(JN)]
    )
    # x2n[n] = x2[:, n-chunk] as [128, JC, 512]
    x2nb = np.stack(
        [_pmajor(x2b[:, n * 512 : (n + 1) * 512], 128, None) for n in range(TN)]
    )
    maps = []
    for c in range(NCORES):
        sl = slice(c * S, (c + 1) * S)
        shard = label_map[sl].astype(NPBF16)
        lmb = np.stack(
            [_pmajor(shard[:, j * 512 : (j + 1) * 512], 128, None) for j in range(JN)]
        )
        maps.append(
            {
                "x1t": _pmajor(np.ascontiguousarray(x1b[sl].T), 128, None),
                "wqt": wqt,
                "wk": wkt,
                "x2t": x2tb,
                "x2n": x2nb,
                "lm": lmb,
                "bq2": bq2,
            }
        )
    return maps


def _run(x1, x2, label_map, Wq, bq, Wk, bk, topk, trace=False):
    x1 = np.asarray(x1, np.float32)
    x2 = np.asarray(x2, np.float32)
    label_map = np.asarray(label_map, np.float32)
    Wq = np.asarray(Wq, np.float32)
    bq = np.asarray(bq, np.float32)
    Wk = np.asarray(Wk, np.float32)
    N1, ITEM = x1.shape
    N2 = x2.shape[0]
    DIM = Wq.shape[0]
    S = N1 // NCORES
    DIMP = ((DIM + 127) // 128) * 128
    nc = _build(S, N2, ITEM, DIMP, math.sqrt(ITEM), float(topk))
    maps = _in_maps(x1, x2, label_map, Wq, bq, Wk, DIMP, S)
    res = run_bass_kernel_spmd(
        nc, maps, list(range(NCORES)), trace=trace, trace_cores=[0] if trace else None
    )
    out = np.concatenate([res.results[c]["y"] for c in range(NCORES)], axis=0)
    return out.astype(np.float32), res


def kernel(x1, x2, label_map, Wq, bq, Wk, bk, topk):
    out, _ = _run(x1, x2, label_map, Wq, bq, Wk, bk, topk)
    return out


# revision 7
# speedup vs baseline: 1.4081x; 1.4081x over previous
"""Sparse-attention kernel for 8 trn2 NeuronCores (Bass/Tile).

Math (reference):
    Q = x1 @ Wq.T + bq                       [N1, DIM]
    K = x2 @ Wk.T + bk                       [N2, DIM]
    scores = (Q @ K.T) / sqrt(ITEM)          [N1, N2]
    e = exp(scores) * label_map
    att = e / (sum_j e + 1e-8) * (sum_j label_map / topk + 1e-8)
    out = att @ x2                           [N1, ITEM]

Key transformations used here:
  * Rows of x1/label_map are sharded across 8 cores (512 rows each).
  * scores = Q @ K.T is reassociated as (Q @ Wk) @ x2.T + (Q @ bk) 1^T.
    The (Q @ bk) term is constant per output row i; it scales both the
    numerator e and the denominator sum(e) by exp(c_i), which cancels in
    the normalization (the +1e-8 epsilon makes this inexact only at the
    ~1e-11 relative level since sum(e) is O(1e3)).  So bk drops out and no
    core ever computes the K projection (saving 33.5 GFLOP/core).
  * The 1/sqrt(ITEM) scale and bq bias are folded into the Q epilogue.
  * The per-row normalization a_i is applied to the final out rows, so the
    unnormalized e.T tiles (built via PE transposes) feed the spmm directly.
  * Matmul operands are bf16 (fp32 PSUM accumulation).  bf16 weights get
    the fast-weight-load path; 4-byte weights serialize LDWEIGHTS with the
    matmul (~+190 ns each).
  * Every DRAM stream is host-rearranged partition-major so each SBUF slab
    loads with ONE fully contiguous DMA (multi-KB lines on both sides), and
    matmul operand tiles are free-dim slices of resident slabs.
"""

import math

import numpy as np

try:
    import concourse.bass as bass
except ImportError:  # fresh interpreter without the boot path
    import sys

    sys.path.insert(0, "/opt/trn_rl_repo")
    import concourse.bass as bass

import ml_dtypes
import concourse.mybir as mybir
import concourse.tile as tile
from concourse import bacc
from concourse.bass_utils import run_bass_kernel_spmd
from concourse.masks import make_identity

NCORES = 8
F32 = mybir.dt.float32
BF16 = mybir.dt.bfloat16
NPBF16 = ml_dtypes.bfloat16


def _build(S, N2, ITEM, DIMP, denom, topk_f):
    """Build the per-core Bass program.

    S     - x1 rows per core (multiple of 128)
    N2    - x2 rows (multiple of 512)
    ITEM  - feature dim (multiple of 512)
    DIMP  - projection dim padded to a multiple of 128
    denom - sqrt(original ITEM)
    """
    IC = S // 128  # output-row chunks
    JC = N2 // 128  # x2-row chunks (spmm contraction)
    JN = N2 // 512  # 512-wide tiles of the scores free dim
    TC = ITEM // 128  # feature chunks (scores contraction)
    TN = ITEM // 512  # 512-wide tiles of the output free dim
    DC = DIMP // 128  # projection-dim chunks
    Exp = mybir.ActivationFunctionType.Exp
    Mult = mybir.AluOpType.mult
    Add = mybir.AluOpType.add
    X = mybir.AxisListType.X

    nc = bacc.Bacc("TRN2", target_bir_lowering=False, debug=False, num_devices=NCORES)
    # All streams are partition-major slabs: [slab_idx, 128, inner...] where
    # the per-partition inner block is contiguous in DRAM.
    x1t = nc.dram_tensor("x1t", [128, TC, S], BF16, kind="ExternalInput")
    wqt = nc.dram_tensor("wqt", [DC, 128, TC, 128], BF16, kind="ExternalInput")
    wk = nc.dram_tensor("wk", [TC, 128, DC, 128], BF16, kind="ExternalInput")
    x2t = nc.dram_tensor("x2t", [JN, 128, TC, 512], BF16, kind="ExternalInput")
    x2n = nc.dram_tensor("x2n", [TN, 128, JC, 512], BF16, kind="ExternalInput")
    lm = nc.dram_tensor("lm", [JN, 128, IC, 512], BF16, kind="ExternalInput")
    bq2 = nc.dram_tensor("bq2", [128, DC], F32, kind="ExternalInput")
    y = nc.dram_tensor("y", [S, ITEM], F32, kind="ExternalOutput")

    with tile.TileContext(nc) as tc:
        with (
            tc.tile_pool(name="big", bufs=1) as big,
            tc.tile_pool(name="persist", bufs=1) as persist,
            tc.tile_pool(name="stream", bufs=4) as stream,
            tc.tile_pool(name="slab", bufs=2) as slabpool,
            tc.tile_pool(name="wq", bufs=2) as wqpool,
            tc.tile_pool(name="wkp", bufs=2) as wkpool,
            tc.tile_pool(name="lmp", bufs=2) as lmpool,
            tc.tile_pool(name="acc", bufs=4, space="PSUM") as accp,
            tc.tile_pool(name="trp", bufs=4, space="PSUM") as trp,
        ):
            ident = persist.tile([128, 128], BF16, tag="ident")
            make_identity(nc, ident[:])
            zbias = persist.tile([128, 1], F32, tag="zbias")
            nc.gpsimd.memset(zbias[:], 0.0)
            bqt = persist.tile([128, DC], F32, tag="bqt")
            nc.sync.dma_start(bqt[:], bq2[:])
            bqs = persist.tile([128, DC], F32, tag="bqs")
            nc.vector.tensor_scalar_mul(bqs[:], bqt[:], 1.0 / denom)

            # phase 1: QT[d, i] = (x1 @ Wq.T + bq) / denom, DIM-major
            x1t_s = big.tile([128, TC, S], BF16, tag="bigA")
            nc.sync.dma_start(x1t_s[:], x1t[:])
            qt_s = persist.tile([128, DC, S], BF16, tag="qt")
            for d in range(DC):
                wsl = wqpool.tile([128, TC, 128], BF16, tag="wq")
                nc.sync.dma_start(wsl[:], wqt[d])
                ps = accp.tile([128, 512], F32, tag="acc")
                for t in range(TC):
                    nc.tensor.matmul(
                        ps[:, :S],
                        wsl[:, t, :],
                        x1t_s[:, t, :],
                        start=(t == 0),
                        stop=(t == TC - 1),
                    )
                nc.vector.tensor_scalar(
                    qt_s[:, d, :], ps[:, :S], 1.0 / denom, bqs[:, d : d + 1],
                    op0=Mult, op1=Add,
                )

            # phase 2: AT[t, i] = sum_d Wk[d, t] * QT[d, i]   (= (Q @ Wk).T)
            at_s = big.tile([128, TC, S], BF16, tag="bigB")
            for t in range(TC):
                wsl = wkpool.tile([128, DC, 128], BF16, tag="wk")
                nc.sync.dma_start(wsl[:], wk[t])
                ps = accp.tile([128, 512], F32, tag="acc")
                for d in range(DC):
                    nc.tensor.matmul(
                        ps[:, :S],
                        wsl[:, d, :],
                        qt_s[:, d, :],
                        start=(d == 0),
                        stop=(d == DC - 1),
                    )
                nc.scalar.copy(at_s[:, t, :], ps[:, :S])

            # phase 3: scores -> exp -> *label -> row-sums -> transpose to eT
            et_s = big.tile([128, JC, IC * 128], BF16, tag="bigA")
            s_parts = persist.tile([128, IC, JN], F32, tag="sparts")
            i_parts = persist.tile([128, IC, JN], F32, tag="iparts")
            for jn in range(JN):
                xsl = slabpool.tile([128, TC, 512], BF16, tag="slab")
                nc.sync.dma_start(xsl[:], x2t[jn])
                lsl = lmpool.tile([128, IC, 512], BF16, tag="lmt")
                nc.sync.dma_start(lsl[:], lm[jn])
                for i in range(IC):
                    ps = accp.tile([128, 512], F32, tag="acc", name=f"ps3_{jn}_{i}")
                    for t in range(TC):
                        nc.tensor.matmul(
                            ps[:],
                            at_s[:, t, i * 128 : (i + 1) * 128],
                            xsl[:, t, :],
                            start=(t == 0),
                            stop=(t == TC - 1),
                        )
                    e = stream.tile([128, 512], BF16, tag="e")
                    nc.scalar.activation(e[:], ps[:], Exp, bias=zbias[:])
                    nc.vector.reduce_sum(
                        i_parts[:, i, jn : jn + 1], lsl[:, i, :], axis=X
                    )
                    nc.vector.tensor_mul(e[:], e[:], lsl[:, i, :])
                    nc.vector.reduce_sum(s_parts[:, i, jn : jn + 1], e[:], axis=X)
                    for jj in range(4):
                        pt = trp.tile([128, 128], BF16, tag="tr")
                        nc.tensor.transpose(
                            pt[:], e[:, jj * 128 : (jj + 1) * 128], ident[:]
                        )
                        nc.scalar.copy(
                            et_s[:, jn * 4 + jj, i * 128 : (i + 1) * 128], pt[:]
                        )

            # a_i = (interactions/topk + 1e-8) / (sum_e + 1e-8)
            s_all = persist.tile([128, IC, 1], F32, tag="sall")
            nc.vector.reduce_sum(s_all[:], s_parts[:], axis=X)
            nc.vector.tensor_scalar_add(s_all[:], s_all[:], 1e-8)
            rec = persist.tile([128, IC, 1], F32, tag="rec")
            nc.vector.reciprocal(rec[:], s_all[:])
            i_all = persist.tile([128, IC, 1], F32, tag="iall")
            nc.vector.reduce_sum(i_all[:], i_parts[:], axis=X)
            nc.vector.tensor_scalar(
                i_all[:], i_all[:], 1.0 / topk_f, 1e-8, op0=Mult, op1=Add
            )
            a_all = persist.tile([128, IC, 1], F32, tag="aall")
            nc.vector.tensor_mul(a_all[:], i_all[:], rec[:])

            # phase 4: out[i, :] = a_i * sum_j eT[j, i] * x2[j, :]
            for n in range(TN):
                xsl = slabpool.tile([128, JC, 512], BF16, tag="slab")
                nc.sync.dma_start(xsl[:], x2n[n])
                for i in range(IC):
                    ps = accp.tile([128, 512], F32, tag="acc", name=f"ps4_{n}_{i}")
                    for j in range(JC):
                        nc.tensor.matmul(
                            ps[:],
                            et_s[:, j, i * 128 : (i + 1) * 128],
                            xsl[:, j, :],
                            start=(j == 0),
                            stop=(j == JC - 1),
                        )
                    o = stream.tile([128, 512], F32, tag="osb")
                    nc.vector.tensor_scalar_mul(o[:], ps[:], a_all[:, i, :])
                    nc.sync.dma_start(
                        y[i * 128 : (i + 1) * 128, n * 512 : (n + 1) * 512], o[:]
                    )

    nc.compile()
    return nc


def _pmajor(a, p, inner):
    """[R, C] with R = nblk*p -> [p, nblk, inner...] partition-major, where
    each partition's inner block is contiguous."""
    R, C = a.shape
    nblk = R // p
    return np.ascontiguousarray(a.reshape(nblk, p, C).transpose(1, 0, 2))


def _in_maps(x1, x2, label_map, Wq, bq, Wk, DIMP, S):
    ITEM = x1.shape[1]
    N2 = x2.shape[0]
    DIM = Wq.shape[0]
    DC = DIMP // 128
    TC = ITEM // 128
    JN = N2 // 512
    TN = ITEM // 512
    JC = N2 // 128
    IC = S // 128

    wqp = np.zeros((DIMP, ITEM), NPBF16)
    wqp[:DIM] = Wq.astype(NPBF16)
    wkp = np.zeros((DIMP, ITEM), NPBF16)
    wkp[:DIM] = Wk.astype(NPBF16)
    bqp = np.zeros((DIMP,), np.float32)
    bqp[:DIM] = bq
    bq2 = np.ascontiguousarray(bqp.reshape(DC, 128).T)

    x1b = x1.astype(NPBF16)
    x2b = x2.astype(NPBF16)
    wqT = np.ascontiguousarray(wqp.T)  # [ITEM, DIMP]
    x2T = np.ascontiguousarray(x2b.T)  # [ITEM, N2]

    # wqt[d] = WqT[:, d-chunk] as [128, TC, 128] partition-major
    wqt = np.stack(
        [_pmajor(wqT[:, d * 128 : (d + 1) * 128], 128, None) for d in range(DC)]
    )
    # wk[t] = Wk_pad[:, t-chunk] as [128, DC, 128]
    wkt = np.stack(
        [_pmajor(wkp[:, t * 128 : (t + 1) * 128], 128, None) for t in range(TC)]
    )
    # x2t[jn] = x2T[:, jn-chunk] as [128, TC, 512]
    x2tb = np.stack(
        [_pmajor(x2T[:, j * 512 : (j + 1) * 512], 128, None) for j in range(JN)]
    )
    # x2n[n] = x2[:, n-chunk] as [128, JC, 512]
    x2nb = np.stack(
        [_pmajor(x2b[:, n * 512 : (n + 1) * 512], 128, None) for n in range(TN)]
    )
    maps = []
    for c in range(NCORES):
        sl = slice(c * S, (c + 1) * S)
        shard = label_map[sl].astype(NPBF16)
        lmb = np.stack(
            [_pmajor(shard[:, j * 512 : (j + 1) * 512], 128, None) for j in range(JN)]
        )
        maps.append(
            {
                "x1t": _pmajor(np.ascontiguousarray(x1b[sl].T), 128, None),
                "wqt": wqt,
                "wk": wkt,
                "x2t": x2tb,
                "x2n": x2nb,
                "lm": lmb,
                "bq2": bq2,
            }
        )
    return maps


def _run(x1, x2, label_map, Wq, bq, Wk, bk, topk, trace=False):
    x1 = np.asarray(x1, np.float32)
    x2 = np.asarray(x2, np.float32)
    label_map = np.asarray(label_map, np.float32)
    Wq = np.asarray(Wq, np.float32)
    bq = np.asarray(bq, np.float32)
    Wk = np.asarray(Wk, np.float32)
    N1, ITEM = x1.shape
    N2 = x2.shape[0]
    DIM = Wq.shape[0]
    S = N1 // NCORES
    DIMP = ((DIM + 127) // 128) * 128
    nc = _build(S, N2, ITEM, DIMP, math.sqrt(ITEM), float(topk))
    maps = _in_maps(x1, x2, label_map, Wq, bq, Wk, DIMP, S)
    res = run_bass_kernel_spmd(
        nc, maps, list(range(NCORES)), trace=trace, trace_cores=[0] if trace else None
    )
    out = np.concatenate([res.results[c]["y"] for c in range(NCORES)], axis=0)
    return out.astype(np.float32), res


def kernel(x1, x2, label_map, Wq, bq, Wk, bk, topk):
    out, _ = _run(x1, x2, label_map, Wq, bq, Wk, bk, topk)
    return out


# revision 9
# speedup vs baseline: 1.4353x; 1.0193x over previous
"""Sparse-attention kernel for 8 trn2 NeuronCores (Bass/Tile).

Math (reference):
    Q = x1 @ Wq.T + bq                       [N1, DIM]
    K = x2 @ Wk.T + bk                       [N2, DIM]
    scores = (Q @ K.T) / sqrt(ITEM)          [N1, N2]
    e = exp(scores) * label_map
    att = e / (sum_j e + 1e-8) * (sum_j label_map / topk + 1e-8)
    out = att @ x2                           [N1, ITEM]

Key transformations used here:
  * Rows of x1/label_map are sharded across 8 cores (512 rows each).
  * scores = Q @ K.T is reassociated as (Q @ Wk) @ x2.T + (Q @ bk) 1^T.
    The (Q @ bk) term is constant per output row i; it scales both the
    numerator e and the denominator sum(e) by exp(c_i), which cancels in
    the normalization (the +1e-8 epsilon makes this inexact only at the
    ~1e-11 relative level since sum(e) is O(1e3)).  So bk drops out and no
    core ever computes the K projection (saving 33.5 GFLOP/core).
  * The 1/sqrt(ITEM) scale and bq bias are folded into the Q epilogue.
  * The per-row normalization a_i is applied to the final out rows, so the
    unnormalized e.T tiles (built via PE transposes) feed the spmm directly.
  * Matmul operands are bf16 (fp32 PSUM accumulation).  bf16 weights get
    the fast-weight-load path; 4-byte weights serialize LDWEIGHTS with the
    matmul (~+190 ns each).
  * Every DRAM stream is host-rearranged partition-major so each SBUF slab
    loads with ONE fully contiguous DMA (multi-KB lines on both sides), and
    matmul operand tiles are free-dim slices of resident slabs.
"""

import math

import numpy as np

try:
    import concourse.bass as bass
except ImportError:  # fresh interpreter without the boot path
    import sys

    sys.path.insert(0, "/opt/trn_rl_repo")
    import concourse.bass as bass

import ml_dtypes
import concourse.mybir as mybir
import concourse.tile as tile
from concourse import bacc
from concourse.bass_utils import run_bass_kernel_spmd
from concourse.masks import make_identity

NCORES = 8
F32 = mybir.dt.float32
BF16 = mybir.dt.bfloat16
NPBF16 = ml_dtypes.bfloat16


def _build(S, N2, ITEM, DIMP, denom, topk_f):
    """Build the per-core Bass program.

    S     - x1 rows per core (multiple of 128)
    N2    - x2 rows (multiple of 512)
    ITEM  - feature dim (multiple of 512)
    DIMP  - projection dim padded to a multiple of 128
    denom - sqrt(original ITEM)
    """
    IC = S // 128  # output-row chunks
    JC = N2 // 128  # x2-row chunks (spmm contraction)
    JN = N2 // 512  # 512-wide tiles of the scores free dim
    TC = ITEM // 128  # feature chunks (scores contraction)
    TN = ITEM // 512  # 512-wide tiles of the output free dim
    DC = DIMP // 128  # projection-dim chunks
    Exp = mybir.ActivationFunctionType.Exp
    Mult = mybir.AluOpType.mult
    Add = mybir.AluOpType.add
    X = mybir.AxisListType.X

    nc = bacc.Bacc("TRN2", target_bir_lowering=False, debug=False, num_devices=NCORES)
    # All streams are partition-major slabs: [slab_idx, 128, inner...] where
    # the per-partition inner block is contiguous in DRAM.
    x1t = nc.dram_tensor("x1t", [128, TC, S], BF16, kind="ExternalInput")
    wqt = nc.dram_tensor("wqt", [DC, 128, TC, 128], BF16, kind="ExternalInput")
    wk = nc.dram_tensor("wk", [TC, 128, DC, 128], BF16, kind="ExternalInput")
    x2t = nc.dram_tensor("x2t", [JN, 128, TC, 512], BF16, kind="ExternalInput")
    x2n = nc.dram_tensor("x2n", [TN, 128, JC, 512], BF16, kind="ExternalInput")
    lm = nc.dram_tensor("lm", [JN, 128, IC, 512], BF16, kind="ExternalInput")
    bq2 = nc.dram_tensor("bq2", [128, DC], F32, kind="ExternalInput")
    y = nc.dram_tensor("y", [S, ITEM], F32, kind="ExternalOutput")

    with tile.TileContext(nc) as tc:
        with (
            tc.tile_pool(name="big", bufs=1) as big,
            tc.tile_pool(name="persist", bufs=1) as persist,
            tc.tile_pool(name="stream", bufs=4) as stream,
            tc.tile_pool(name="slab", bufs=8) as slabpool,
            tc.tile_pool(name="wq", bufs=2) as wqpool,
            tc.tile_pool(name="wkp", bufs=2) as wkpool,
            tc.tile_pool(name="lmp", bufs=2) as lmpool,
            tc.tile_pool(name="acc", bufs=4, space="PSUM") as accp,
            tc.tile_pool(name="trp", bufs=4, space="PSUM") as trp,
        ):
            ident = persist.tile([128, 128], BF16, tag="ident")
            make_identity(nc, ident[:])
            zbias = persist.tile([128, 1], F32, tag="zbias")
            nc.gpsimd.memset(zbias[:], 0.0)
            bqt = persist.tile([128, DC], F32, tag="bqt")
            nc.sync.dma_start(bqt[:], bq2[:])
            bqs = persist.tile([128, DC], F32, tag="bqs")
            nc.vector.tensor_scalar_mul(bqs[:], bqt[:], 1.0 / denom)

            # phase 1: QT[d, i] = (x1 @ Wq.T + bq) / denom, DIM-major
            x1t_s = big.tile([128, TC, S], BF16, tag="bigA")
            nq = min(8, TC)
            qw = TC // nq
            for q in range(nq):
                nc.sync.dma_start(
                    x1t_s[:, q * qw : (q + 1) * qw, :], x1t[:, q * qw : (q + 1) * qw, :]
                )
            qt_s = persist.tile([128, DC, S], BF16, tag="qt")
            for d in range(DC):
                wsl = wqpool.tile([128, TC, 128], BF16, tag="wq")
                nc.sync.dma_start(wsl[:], wqt[d])
                ps = accp.tile([128, 512], F32, tag="acc")
                for t in range(TC):
                    nc.tensor.matmul(
                        ps[:, :S],
                        wsl[:, t, :],
                        x1t_s[:, t, :],
                        start=(t == 0),
                        stop=(t == TC - 1),
                    )
                nc.vector.tensor_scalar(
                    qt_s[:, d, :], ps[:, :S], 1.0 / denom, bqs[:, d : d + 1],
                    op0=Mult, op1=Add,
                )

            # phase 2: AT[t, i] = sum_d Wk[d, t] * QT[d, i]   (= (Q @ Wk).T)
            at_s = big.tile([128, TC, S], BF16, tag="bigB")
            for t in range(TC):
                wsl = wkpool.tile([128, DC, 128], BF16, tag="wk")
                nc.sync.dma_start(wsl[:], wk[t])
                ps = accp.tile([128, 512], F32, tag="acc")
                for d in range(DC):
                    nc.tensor.matmul(
                        ps[:, :S],
                        wsl[:, d, :],
                        qt_s[:, d, :],
                        start=(d == 0),
                        stop=(d == DC - 1),
                    )
                nc.scalar.copy(at_s[:, t, :], ps[:, :S])

            # phase 3: scores -> exp -> *label -> row-sums -> transpose to eT
            et_s = big.tile([128, JC, IC * 128], BF16, tag="bigA")
            s_parts = persist.tile([128, IC, JN], F32, tag="sparts")
            i_parts = persist.tile([128, IC, JN], F32, tag="iparts")
            QW = TC // 4
            for jn in range(JN):
                xq = []
                for q in range(4):
                    xs = slabpool.tile(
                        [128, QW, 512], BF16, tag="slab", name=f"x2t_{jn}_{q}"
                    )
                    nc.sync.dma_start(xs[:], x2t[jn, :, q * QW : (q + 1) * QW, :])
                    xq.append(xs)
                lsl = lmpool.tile([128, IC, 512], BF16, tag="lmt")
                nc.sync.dma_start(lsl[:], lm[jn])
                for i in range(IC):
                    ps = accp.tile([128, 512], F32, tag="acc", name=f"ps3_{jn}_{i}")
                    for t in range(TC):
                        nc.tensor.matmul(
                            ps[:],
                            at_s[:, t, i * 128 : (i + 1) * 128],
                            xq[t // QW][:, t % QW, :],
                            start=(t == 0),
                            stop=(t == TC - 1),
                        )
                    e = stream.tile([128, 512], BF16, tag="e")
                    nc.scalar.activation(e[:], ps[:], Exp, bias=zbias[:])
                    nc.vector.reduce_sum(
                        i_parts[:, i, jn : jn + 1], lsl[:, i, :], axis=X
                    )
                    nc.vector.tensor_mul(e[:], e[:], lsl[:, i, :])
                    nc.vector.reduce_sum(s_parts[:, i, jn : jn + 1], e[:], axis=X)
                    for jj in range(4):
                        pt = trp.tile([128, 128], BF16, tag="tr")
                        nc.tensor.transpose(
                            pt[:], e[:, jj * 128 : (jj + 1) * 128], ident[:]
                        )
                        nc.scalar.copy(
                            et_s[:, jn * 4 + jj, i * 128 : (i + 1) * 128], pt[:]
                        )

            # a_i = (interactions/topk + 1e-8) / (sum_e + 1e-8)
            s_all = persist.tile([128, IC, 1], F32, tag="sall")
            nc.vector.reduce_sum(s_all[:], s_parts[:], axis=X)
            nc.vector.tensor_scalar_add(s_all[:], s_all[:], 1e-8)
            rec = persist.tile([128, IC, 1], F32, tag="rec")
            nc.vector.reciprocal(rec[:], s_all[:])
            i_all = persist.tile([128, IC, 1], F32, tag="iall")
            nc.vector.reduce_sum(i_all[:], i_parts[:], axis=X)
            nc.vector.tensor_scalar(
                i_all[:], i_all[:], 1.0 / topk_f, 1e-8, op0=Mult, op1=Add
            )
            a_all = persist.tile([128, IC, 1], F32, tag="aall")
            nc.vector.tensor_mul(a_all[:], i_all[:], rec[:])

            # phase 4: out[i, :] = a_i * sum_j eT[j, i] * x2[j, :]
            QJ = JC // 4
            for n in range(TN):
                xq = []
                for q in range(4):
                    xs = slabpool.tile(
                        [128, QJ, 512], BF16, tag="slab", name=f"x2n_{n}_{q}"
                    )
                    nc.sync.dma_start(xs[:], x2n[n, :, q * QJ : (q + 1) * QJ, :])
                    xq.append(xs)
                for i in range(IC):
                    ps = accp.tile([128, 512], F32, tag="acc", name=f"ps4_{n}_{i}")
                    for j in range(JC):
                        nc.tensor.matmul(
                            ps[:],
                            et_s[:, j, i * 128 : (i + 1) * 128],
                            xq[j // QJ][:, j % QJ, :],
                            start=(j == 0),
                            stop=(j == JC - 1),
                        )
                    o = stream.tile([128, 512], F32, tag="osb")
                    nc.vector.tensor_scalar_mul(o[:], ps[:], a_all[:, i, :])
                    nc.sync.dma_start(
                        y[i * 128 : (i + 1) * 128, n * 512 : (n + 1) * 512], o[:]
                    )

    nc.compile()
    return nc


def _pmajor(a, p, inner):
    """[R, C] with R = nblk*p -> [p, nblk, inner...] partition-major, where
    each partition's inner block is contiguous."""
    R, C = a.shape
    nblk = R // p
    return np.ascontiguousarray(a.reshape(nblk, p, C).transpose(1, 0, 2))


def _in_maps(x1, x2, label_map, Wq, bq, Wk, DIMP, S):
    ITEM = x1.shape[1]
    N2 = x2.shape[0]
    DIM = Wq.shape[0]
    DC = DIMP // 128
    TC = ITEM // 128
    JN = N2 // 512
    TN = ITEM // 512
    JC = N2 // 128
    IC = S // 128

    wqp = np.zeros((DIMP, ITEM), NPBF16)
    wqp[:DIM] = Wq.astype(NPBF16)
    wkp = np.zeros((DIMP, ITEM), NPBF16)
    wkp[:DIM] = Wk.astype(NPBF16)
    bqp = np.zeros((DIMP,), np.float32)
    bqp[:DIM] = bq
    bq2 = np.ascontiguousarray(bqp.reshape(DC, 128).T)

    x1b = x1.astype(NPBF16)
    x2b = x2.astype(NPBF16)
    wqT = np.ascontiguousarray(wqp.T)  # [ITEM, DIMP]
    x2T = np.ascontiguousarray(x2b.T)  # [ITEM, N2]

    # wqt[d] = WqT[:, d-chunk] as [128, TC, 128] partition-major
    wqt = np.stack(
        [_pmajor(wqT[:, d * 128 : (d + 1) * 128], 128, None) for d in range(DC)]
    )
    # wk[t] = Wk_pad[:, t-chunk] as [128, DC, 128]
    wkt = np.stack(
        [_pmajor(wkp[:, t * 128 : (t + 1) * 128], 128, None) for t in range(TC)]
    )
    # x2t[jn] = x2T[:, jn-chunk] as [128, TC, 512]
    x2tb = np.stack(
        [_pmajor(x2T[:, j * 512 : (j + 1) * 512], 128, None) for j in range(JN)]
    )
    # x2n[n] = x2[:, n-chunk] as [128, JC, 512]
    x2nb = np.stack(
        [_pmajor(x2b[:, n * 512 : (n + 1) * 512], 128, None) for n in range(TN)]
    )
    maps = []
    for c in range(NCORES):
        sl = slice(c * S, (c + 1) * S)
        shard = label_map[sl].astype(NPBF16)
        lmb = np.stack(
            [_pmajor(shard[:, j * 512 : (j + 1) * 512], 128, None) for j in range(JN)]
        )
        maps.append(
            {
                "x1t": _pmajor(np.ascontiguousarray(x1b[sl].T), 128, None),
                "wqt": wqt,
                "wk": wkt,
                "x2t": x2tb,
                "x2n": x2nb,
                "lm": lmb,
                "bq2": bq2,
            }
        )
    return maps


def _run(x1, x2, label_map, Wq, bq, Wk, bk, topk, trace=False):
    x1 = np.asarray(x1, np.float32)
    x2 = np.asarray(x2, np.float32)
    label_map = np.asarray(label_map, np.float32)
    Wq = np.asarray(Wq, np.float32)
    bq = np.asarray(bq, np.float32)
    Wk = np.asarray(Wk, np.float32)
    N1, ITEM = x1.shape
    N2 = x2.shape[0]
    DIM = Wq.shape[0]
    S = N1 // NCORES
    DIMP = ((DIM + 127) // 128) * 128
    nc = _build(S, N2, ITEM, DIMP, math.sqrt(ITEM), float(topk))
    maps = _in_maps(x1, x2, label_map, Wq, bq, Wk, DIMP, S)
    res = run_bass_kernel_spmd(
        nc, maps, list(range(NCORES)), trace=trace, trace_cores=[0] if trace else None
    )
    out = np.concatenate([res.results[c]["y"] for c in range(NCORES)], axis=0)
    return out.astype(np.float32), res


def kernel(x1, x2, label_map, Wq, bq, Wk, bk, topk):
    out, _ = _run(x1, x2, label_map, Wq, bq, Wk, bk, topk)
    return out
